# revision 19
# baseline (speedup 1.0000x reference)
"""Trainium2 Bass kernel for nn_BeamSearchDecoder (B=32, K=3, H=512, E=256,
V=32000, 32 decode steps), SPMD over 8 NeuronCores.

Key observation (verified against the reference): all K=3 beams start from an
identical state (h0 repeated, same START token, zero scores) and jax.lax.top_k
breaks ties by lower index, so the beam search is exactly greedy decoding with
every beam identical at every step (bitwise).  The kernel therefore runs a
greedy GRU decoder over 32 batch rows and the host replicates beams / builds
the one-hot output.

Distribution: the output projection W_out (32000x512 fp32 = 65.5 MB) is
sharded over the 8 cores by vocab (4000 rows each, SBUF-resident).  Each step
every core computes its logits shard, local top-2 candidates (+ sum-exp for
the log-softmax normalizer), and the 8 per-core candidate rows are combined
with a small AllGather; every core then picks the same next token.

Precision scheme: the big logits matmul runs in float32r (4x faster PE mode,
~1e-4 abs error).  That is accurate enough to FIND the top-2 candidates but
not to ORDER near-ties faithfully vs the fp32 reference, so after the
exchange the two global finalists per row are recomputed exactly (fp32 dot on
the vector engine against W_out rows gathered from DRAM) and the winner is
chosen from the exact values.  The GRU itself stays fp32 so the hidden state
tracks the reference bit-for-bit-ish (~1e-7).

The GRU input projection is folded into a host-precomputed table
  gi_table[v] = relu(emb[v]) @ W_ih.T + b_ih + [b_hh_r, b_hh_z, 0]
gathered per step with an indirect DMA (the n-gate b_hh part must stay inside
the r*(...) term, so it is added separately on device).
"""

import sys

import numpy as np

for _p in ("/opt/trn_rl_repo", "/root/.axon_site/_ro/trn_rl_repo"):
    if _p not in sys.path:
        sys.path.append(_p)

import concourse.bass as bass
import concourse.mybir as mybir
import concourse.tile as tile
from concourse import bacc
from concourse.bass_utils import run_bass_kernel_spmd
from concourse.masks import make_identity

F32 = mybir.dt.float32
F32R = mybir.dt.float32r
I32 = mybir.dt.int32
U32 = mybir.dt.uint32
U8 = mybir.dt.uint8
AF = mybir.ActivationFunctionType
OP = mybir.AluOpType
AX = mybir.AxisListType

P = 128
NCORES = 8
B = 32          # batch
K = 3           # beams (degenerate/identical)
H = 512
E = 256
V = 32000
STEPS = 32
START = 1
VSH = V // NCORES          # 4000 vocab per core
QW = VSH // 4              # 1000 per quarter-partition-group
KSUB = H // P              # 4
BIG = 1.0e9
PAY = 5 * B                # payload row: v1 | i1 | v2 | i2 | se


def _build_nc(steps=STEPS, use_cc=True):
    nc = bacc.Bacc("TRN2", target_bir_lowering=False, debug=False,
                   num_devices=NCORES)

    gi_table = nc.dram_tensor("gi_table", [V, 3 * H], F32, kind="ExternalInput")
    w_hht = nc.dram_tensor("w_hht", [P, KSUB, 3 * H], F32, kind="ExternalInput")
    b_hhn = nc.dram_tensor("b_hhn", [B, H], F32, kind="ExternalInput")
    wo = nc.dram_tensor("wo", [P, 16, QW], F32, kind="ExternalInput")
    bo = nc.dram_tensor("bo", [B, 4, QW], F32, kind="ExternalInput")
    w_full = nc.dram_tensor("w_full", [V, H], F32, kind="ExternalInput")
    b_outc = nc.dram_tensor("b_outc", [V, 1], F32, kind="ExternalInput")
    h0t = nc.dram_tensor("h0t", [P, KSUB, B], F32, kind="ExternalInput")
    h0row = nc.dram_tensor("h0row", [B, H], F32, kind="ExternalInput")
    vocoff = nc.dram_tensor("vocoff", [P, 1], F32, kind="ExternalInput")
    hbidx = nc.dram_tensor("hbidx", [2 * B, 1], I32, kind="ExternalInput")

    steps_out = nc.dram_tensor("steps_out", [STEPS, B, 8], F32,
                               kind="ExternalOutput")
    toks_out = nc.dram_tensor("toks_out", [STEPS, B, 1], F32,
                              kind="ExternalOutput")
    h_out = nc.dram_tensor("h_out", [B, H], F32, kind="ExternalOutput")

    with tile.TileContext(nc) as tc:
        with tc.tile_pool(name="const", bufs=1) as cp, \
             tc.tile_pool(name="sb", bufs=2) as sb, \
             tc.tile_pool(name="ps", bufs=1, space="PSUM") as ps, \
             tc.tile_pool(name="dr", bufs=2, space="DRAM") as dr:

            # ---- resident constants ----
            w_hht_sb = cp.tile([P, KSUB, 3 * H], F32)
            nc.sync.dma_start(w_hht_sb[:], w_hht[:])
            # wo loaded via staging chunks and rounded into an F32R tile
            wo_r = cp.tile([P, 16, QW], F32R)
            for i in range(16):
                wstage = sb.tile([P, QW], F32, tag="wstage")
                nc.sync.dma_start(wstage[:], wo[:, i, :])
                nc.vector.tensor_copy(wo_r[:, i, :], wstage[:])
            bo_sb = cp.tile([B, 4, QW], F32)
            nc.sync.dma_start(bo_sb[:], bo[:])
            b_hhn_sb = cp.tile([B, H], F32)
            nc.sync.dma_start(b_hhn_sb[:], b_hhn[:])
            vocoff_sb = cp.tile([P, 1], F32)
            nc.sync.dma_start(vocoff_sb[:], vocoff[:])
            hbidx_sb = cp.tile([2 * B, 1], I32)
            nc.sync.dma_start(hbidx_sb[:], hbidx[:])
            ident = cp.tile([P, P], F32)
            make_identity(nc, ident[:])
            big_sb = cp.tile([P, 1], F32)
            nc.vector.memset(big_sb[:], BIG)
            nbig_sb = cp.tile([P, 1], F32)
            nc.vector.memset(nbig_sb[:], -BIG)

            # ---- state ----
            hT = cp.tile([P, KSUB, B], F32, name="hT0")
            nc.sync.dma_start(hT[:], h0t[:])
            hrow = cp.tile([B, H], F32, name="hrow0")
            nc.sync.dma_start(hrow[:], h0row[:])
            tok = cp.tile([B, 1], I32, name="tok0")
            nc.vector.memset(tok[:], START)

            for t in range(steps):
                # 1. gather gi = gi_table[tok]  -> [B, 3H]
                gi = sb.tile([B, 3 * H], F32, tag="gi")
                nc.gpsimd.indirect_dma_start(
                    out=gi[:], out_offset=None,
                    in_=gi_table[:],
                    in_offset=bass.IndirectOffsetOnAxis(ap=tok[:, :1], axis=0),
                )

                # 2. gh matmuls: RZ [B,1024], HN [B,512] (fp32)
                rz_ps = ps.tile([B, 2 * H], F32, tag="rz")
                hn_ps = ps.tile([B, H], F32, tag="hn")
                for ch in range(2):
                    for k in range(KSUB):
                        nc.tensor.matmul(
                            rz_ps[:, ch * H:(ch + 1) * H],
                            lhsT=hT[:, k, :],
                            rhs=w_hht_sb[:, k, ch * H:(ch + 1) * H],
                            start=(k == 0), stop=(k == KSUB - 1))
                for k in range(KSUB):
                    nc.tensor.matmul(
                        hn_ps[:],
                        lhsT=hT[:, k, :],
                        rhs=w_hht_sb[:, k, 2 * H:3 * H],
                        start=(k == 0), stop=(k == KSUB - 1))

                # 3. r,z = sigmoid(gh_rz + gi_rz)
                rzs = sb.tile([B, 2 * H], F32, tag="rzs")
                nc.vector.tensor_add(rzs[:], rz_ps[:], gi[:, :2 * H])
                sig = sb.tile([B, 2 * H], F32, tag="sig")
                nc.scalar.activation(sig[:], rzs[:], AF.Sigmoid)

                # 4. n = tanh(gi_n + r * (gh_n + b_hh_n))
                hn2 = sb.tile([B, H], F32, tag="hn2")
                nc.vector.tensor_add(hn2[:], hn_ps[:], b_hhn_sb[:])
                nc.vector.tensor_mul(hn2[:], hn2[:], sig[:, :H])
                nc.vector.tensor_add(hn2[:], hn2[:], gi[:, 2 * H:3 * H])
                n_sb = sb.tile([B, H], F32, tag="n")
                nc.scalar.activation(n_sb[:], hn2[:], AF.Tanh)

                # 5. h_new = n + z * (h - n)
                hrow_new = sb.tile([B, H], F32, tag="hrow")
                nc.vector.tensor_sub(hrow_new[:], hrow[:], n_sb[:])
                nc.vector.tensor_mul(hrow_new[:], hrow_new[:], sig[:, H:2 * H])
                nc.vector.tensor_add(hrow_new[:], hrow_new[:], n_sb[:])

                # stage h to DRAM for the exact-recompute gather
                h_dram = dr.tile([B, H], F32, tag="hdram")
                nc.sync.dma_start(h_dram[:], hrow_new[:])

                # 6. transpose h_new -> hT_new [128, 4, B]
                ht_ps = ps.tile([P, KSUB * B], F32, tag="htp")
                for k in range(KSUB):
                    nc.tensor.transpose(ht_ps[:, k * B:(k + 1) * B],
                                        hrow_new[:, k * P:(k + 1) * P],
                                        ident[:B, :B])
                hT_new = sb.tile([P, KSUB, B], F32, tag="hT")
                nc.vector.tensor_copy(hT_new[:].rearrange("p k b -> p (k b)"),
                                      ht_ps[:])
                hT_r = sb.tile([P, KSUB, B], F32R, tag="hTr")
                nc.vector.tensor_copy(hT_r[:].rearrange("p k b -> p (k b)"),
                                      ht_ps[:])

                # 7. logits shard in fp32r, 4 serial chunks of 1000
                #    chunk c covers vocab [c*1000, (c+1)*1000) of this shard
                cand_v = sb.tile([B, 8], F32, tag="candv")
                cand_if = sb.tile([B, 8], F32, tag="candif")
                seall = sb.tile([B, 4], F32, tag="seall")
                for c in range(4):
                    lgc = ps.tile([B, 1024], F32, tag="lgc")
                    for c0, c1 in ((0, H), (H, QW)):
                        for k in range(KSUB):
                            nc.tensor.matmul(
                                lgc[:, c0:c1],
                                lhsT=hT_r[:, k, :],
                                rhs=wo_r[:, c * KSUB + k, c0:c1],
                                start=(k == 0), stop=(k == KSUB - 1))
                    nc.vector.tensor_add(lgc[:, :QW], lgc[:, :QW],
                                         bo_sb[:, c, :])
                    v8c = sb.tile([B, 8], F32, tag="v8c")
                    nc.vector.max(v8c[:], lgc[:, :QW])
                    i8c = sb.tile([B, 8], U32, tag="i8c")
                    nc.vector.max_index(i8c[:], v8c[:], lgc[:, :QW])
                    esc = sb.tile([B, QW], F32, tag="esc")
                    nc.scalar.activation(esc[:], lgc[:, :QW], AF.Exp,
                                         accum_out=seall[:, c:c + 1])
                    nc.vector.tensor_copy(cand_v[:, 2 * c:2 * c + 2],
                                          v8c[:, 0:2])
                    nc.vector.tensor_copy(cand_if[:, 2 * c:2 * c + 2],
                                          i8c[:, 0:2])
                    if c:
                        nc.vector.tensor_scalar(
                            cand_if[:, 2 * c:2 * c + 2],
                            cand_if[:, 2 * c:2 * c + 2],
                            float(c * QW), None, op0=OP.add)
                nc.vector.tensor_scalar(cand_if[:], cand_if[:],
                                        vocoff_sb[:B, 0:1], None, op0=OP.add)

                # 8. payload row per batch: v1 i1 v2 i2 se 0 0 0
                pay5 = sb.tile([B, 8], F32, tag="pay5")
                nc.vector.memset(pay5[:], 0.0)
                nc.vector.tensor_reduce(out=pay5[:, 4:5], in_=seall[:],
                                        op=OP.add, axis=AX.X)

                def top2_cols(valv, idxv, fshape, axis, out, cv1, ci1, cv2,
                              ci2, tg):
                    # top-2 by (value desc, ties lowest idx) over free axes
                    bc = lambda apc: apc[:B, 0:1].to_broadcast(fshape)                         if len(fshape) == 2 else                         apc[:B, 0:1, None].to_broadcast(fshape)
                    bco = lambda apo: apo.to_broadcast(fshape)                         if len(fshape) == 2 else                         apo[:, :, None].to_broadcast(fshape)
                    nc.vector.tensor_reduce(out=out[:, cv1:cv1 + 1],
                                            in_=valv, op=OP.max, axis=axis)
                    eqv = sb.tile(list(fshape), U8, tag=f"eqv{tg}")
                    nc.vector.tensor_tensor(eqv[:], valv,
                                            bco(out[:, cv1:cv1 + 1]),
                                            OP.is_equal)
                    i1s = sb.tile(list(fshape), F32, tag=f"i1s{tg}")
                    nc.vector.select(i1s[:], eqv[:], idxv, bc(big_sb))
                    nc.vector.tensor_reduce(out=out[:, ci1:ci1 + 1],
                                            in_=i1s[:], op=OP.min, axis=axis)
                    eqi = sb.tile(list(fshape), U8, tag=f"eqi{tg}")
                    nc.vector.tensor_tensor(eqi[:], idxv,
                                            bco(out[:, ci1:ci1 + 1]),
                                            OP.is_equal)
                    nc.vector.tensor_tensor(eqi[:], eqi[:], eqv[:],
                                            OP.logical_and)
                    vm = sb.tile(list(fshape), F32, tag=f"vm{tg}")
                    nc.vector.select(vm[:], eqi[:], bc(nbig_sb), valv)
                    nc.vector.tensor_reduce(out=out[:, cv2:cv2 + 1],
                                            in_=vm[:], op=OP.max, axis=axis)
                    eq2v = sb.tile(list(fshape), U8, tag=f"eq2v{tg}")
                    nc.vector.tensor_tensor(eq2v[:], vm[:],
                                            bco(out[:, cv2:cv2 + 1]),
                                            OP.is_equal)
                    i2s = sb.tile(list(fshape), F32, tag=f"i2s{tg}")
                    nc.vector.select(i2s[:], eq2v[:], idxv, bc(big_sb))
                    nc.vector.tensor_reduce(out=out[:, ci2:ci2 + 1],
                                            in_=i2s[:], op=OP.min, axis=axis)

                top2_cols(cand_v[:], cand_if[:], (B, 8), AX.X,
                          pay5, 0, 1, 2, 3, "q")

                # own-core per-step record -> host
                nc.sync.dma_start(steps_out[t], pay5[:])

                if t == steps - 1:
                    break   # host does the final-step selection itself

                # 9. exchange candidate rows
                cc_in = dr.tile([B, 8], F32, tag="ccin")
                cc_out = dr.tile([NCORES, B, 8], F32,
                                 addr_space="Shared" if use_cc else "Local",
                                 tag="ccout")
                nc.sync.dma_start(cc_in[:], pay5[:])
                if use_cc:
                    nc.gpsimd.collective_compute(
                        "AllGather", OP.bypass,
                        replica_groups=[list(range(NCORES))],
                        ins=[cc_in[:].opt()], outs=[cc_out[:].opt()],
                    )
                else:  # timing-only variant: fake the gather locally
                    nc.sync.dma_start(
                        cc_out[:].rearrange("c b f -> (c b) f"),
                        cc_in[None, :, :].to_broadcast(
                            [NCORES, B, 8]).rearrange("c b f -> (c b) f"))
                recv = sb.tile([B, NCORES, 8], F32, tag="recv")
                nc.sync.dma_start(recv[:],
                                  cc_out[:].rearrange("c b f -> b c f"))

                # 10. global top-2 by fp32r value over 16 candidates
                v4 = recv[:, :, 0:4].rearrange("b c (m vi) -> b c m vi", m=2)
                valg = v4[:, :, :, 0]          # [B, 8, 2]
                idxg = v4[:, :, :, 1]
                gpay = sb.tile([B, 4], F32, tag="gpay")   # gv1 gi1 gv2 gi2
                top2_cols(valg, idxg, (B, NCORES, 2), AX.XY,
                          gpay, 0, 1, 2, 3, "g")

                # 11. exact fp32 recompute of the two finalists
                cand_i = sb.tile([2 * B, 1], I32, tag="candi")
                nc.vector.tensor_copy(cand_i[0:B], gpay[:, 1:2])
                nc.vector.tensor_copy(cand_i[B:2 * B], gpay[:, 3:4])
                wcand = sb.tile([2 * B, H], F32, tag="wcand")
                nc.gpsimd.indirect_dma_start(
                    out=wcand[:], out_offset=None, in_=w_full[:],
                    in_offset=bass.IndirectOffsetOnAxis(ap=cand_i[:, :1],
                                                        axis=0))
                bcand = sb.tile([2 * B, 1], F32, tag="bcand")
                nc.gpsimd.indirect_dma_start(
                    out=bcand[:], out_offset=None, in_=b_outc[:],
                    in_offset=bass.IndirectOffsetOnAxis(ap=cand_i[:, :1],
                                                        axis=0))
                hcand = sb.tile([2 * B, H], F32, tag="hcand")
                nc.gpsimd.indirect_dma_start(
                    out=hcand[:], out_offset=None, in_=h_dram[:],
                    in_offset=bass.IndirectOffsetOnAxis(ap=hbidx_sb[:, :1],
                                                        axis=0))
                nc.vector.tensor_mul(wcand[:], wcand[:], hcand[:])
                vex = sb.tile([2 * B, 1], F32, tag="vex")
                nc.vector.tensor_reduce(out=vex[:], in_=wcand[:],
                                        op=OP.add, axis=AX.X)
                nc.vector.tensor_add(vex[:], vex[:], bcand[:])
                # realign (m,b) rows -> per-b columns via a DRAM bounce
                vex_d = dr.tile([2 * B, 1], F32, tag="vexd")
                nc.sync.dma_start(vex_d[:], vex[:])
                vexb = sb.tile([B, 2], F32, tag="vexb")
                nc.sync.dma_start(vexb[:],
                                  vex_d[:].rearrange("(m b) o -> b (m o)",
                                                     m=2))

                # 12. winner: cand2 iff v2 > v1 or (v2 == v1 and i2 < i1)
                gtm = sb.tile([B, 1], U8, tag="gtm")
                nc.vector.tensor_tensor(gtm[:], vexb[:, 1:2], vexb[:, 0:1],
                                        OP.is_gt)
                eqm = sb.tile([B, 1], U8, tag="eqm")
                nc.vector.tensor_tensor(eqm[:], vexb[:, 1:2], vexb[:, 0:1],
                                        OP.is_equal)
                ltm = sb.tile([B, 1], U8, tag="ltm")
                nc.vector.tensor_tensor(ltm[:], gpay[:, 3:4], gpay[:, 1:2],
                                        OP.is_lt)
                nc.vector.tensor_tensor(eqm[:], eqm[:], ltm[:],
                                        OP.logical_and)
                nc.vector.tensor_tensor(gtm[:], gtm[:], eqm[:],
                                        OP.logical_or)
                tokf = sb.tile([B, 1], F32, tag="tokf")
                nc.vector.select(tokf[:], gtm[:], gpay[:, 3:4], gpay[:, 1:2])
                nc.sync.dma_start(toks_out[t], tokf[:])
                tok_new = sb.tile([B, 1], I32, tag="tok")
                nc.vector.tensor_copy(tok_new[:], tokf[:])

                hT, hrow, tok = hT_new, hrow_new, tok_new

            nc.sync.dma_start(h_out[:], hrow_new[:])

    nc.compile()
    return nc


_NC_CACHE = None


def _get_nc():
    global _NC_CACHE
    if _NC_CACHE is None:
        _NC_CACHE = _build_nc()
    return _NC_CACHE


def _host_prep(encoder_hidden, emb, W_ih, W_hh, b_ih, b_hh, W_out, b_out):
    emb = np.asarray(emb, np.float32)
    W_ih = np.asarray(W_ih, np.float32)
    W_hh = np.asarray(W_hh, np.float32)
    b_ih = np.asarray(b_ih, np.float32)
    b_hh = np.asarray(b_hh, np.float32)
    W_out = np.ascontiguousarray(np.asarray(W_out, np.float32))
    b_out = np.asarray(b_out, np.float32)
    h0 = np.asarray(encoder_hidden, np.float32)

    bias = b_ih.copy()
    bias[:2 * H] += b_hh[:2 * H]
    gi_table = np.maximum(emb, 0.0).astype(np.float32) @ W_ih.T + bias
    gi_table = np.ascontiguousarray(gi_table, np.float32)

    w_hht = np.ascontiguousarray(
        W_hh.T.reshape(KSUB, P, 3 * H).transpose(1, 0, 2), np.float32)
    b_hhn = np.ascontiguousarray(
        np.broadcast_to(b_hh[2 * H:], (B, H)), np.float32)
    h0t = np.ascontiguousarray(
        h0.T.reshape(KSUB, P, B).transpose(1, 0, 2), np.float32)

    common = {
        "gi_table": gi_table,
        "w_hht": w_hht,
        "b_hhn": b_hhn,
        "w_full": W_out,
        "b_outc": np.ascontiguousarray(b_out[:, None]),
        "h0t": h0t,
        "h0row": np.ascontiguousarray(h0, np.float32),
        "hbidx": np.ascontiguousarray(
            (np.arange(2 * B) % B).astype(np.int32)[:, None]),
    }
    in_maps = []
    for c in range(NCORES):
        wc = W_out[c * VSH:(c + 1) * VSH]                      # [4000, 512]
        wo = np.ascontiguousarray(
            wc.T.reshape(KSUB, P, 4, QW).transpose(1, 2, 0, 3)
            .reshape(P, 16, QW), np.float32)
        bc = b_out[c * VSH:(c + 1) * VSH].reshape(4, QW)
        bo = np.ascontiguousarray(
            np.broadcast_to(bc[None], (B, 4, QW)), np.float32)
        vocoff = np.full((P, 1), float(c * VSH), np.float32)
        m = dict(common)
        m.update({"wo": wo, "bo": bo,
                  "vocoff": np.ascontiguousarray(vocoff)})
        in_maps.append(m)
    return in_maps


def _host_finish(results, W_out, b_out):
    """Rebuild (decoded, h, scores) from per-core per-step candidates."""
    W_out = np.asarray(W_out, np.float64)
    b_out = np.asarray(b_out, np.float64)
    h_final = results[0]["h_out"]                          # [B, H] f32
    pays = np.stack([r["steps_out"] for r in results])     # [C, T, B, 8]
    v1 = pays[:, :, :, 0].astype(np.float64)               # [C, T, B]
    i1 = pays[:, :, :, 1].astype(np.int64)
    v2 = pays[:, :, :, 2].astype(np.float64)
    i2 = pays[:, :, :, 3].astype(np.int64)
    sume = pays[:, :, :, 4].astype(np.float64)
    toks_dev = results[0]["toks_out"][:, :, 0].astype(np.int64)  # [T, B]

    cvals = np.concatenate([v1, v2], axis=0)               # [2C, T, B]
    cidx = np.concatenate([i1, i2], axis=0)

    tok = np.zeros((STEPS, B), np.int64)
    tok[:STEPS - 1] = toks_dev[:STEPS - 1]
    # final step: select among the exchanged candidates with exact math
    t = STEPS - 1
    for b in range(B):
        ci = cidx[:, t, b]
        ex = W_out[ci] @ h_final[b].astype(np.float64) + b_out[ci]
        order = np.lexsort((ci, -ex))
        tok[t, b] = ci[order[0]]

    # scores: fp32r candidate value of the chosen token each step
    lse = np.log(sume.sum(axis=0))                         # [T, B]
    hit = (cidx == tok[None]).astype(np.float64)           # [2C, T, B]
    vchosen = (cvals * hit).sum(axis=0)
    cum = (vchosen - lse).sum(axis=0)                      # [B]

    seqs = np.full((B, STEPS + 1), START, np.int64)
    seqs[:, 1:] = tok.T                                    # greedy: old = id
    decoded = np.zeros((B, STEPS + 1, V), np.float32)
    bi = np.repeat(np.arange(B), STEPS + 1)
    ti = np.tile(np.arange(STEPS + 1), B)
    decoded[bi, ti, seqs.reshape(-1)] = 1.0

    h = np.repeat(h_final[:, None, :], K, axis=1).astype(np.float32)
    scores = np.repeat(cum.astype(np.float32)[:, None], K, axis=1)
    return decoded, h, scores


def kernel(encoder_outputs, encoder_hidden, emb, W_ih, W_hh, b_ih, b_hh,
           W_out, b_out):
    nc = _get_nc()
    in_maps = _host_prep(encoder_hidden, emb, W_ih, W_hh, b_ih, b_hh,
                         W_out, b_out)
    res = run_bass_kernel_spmd(nc, in_maps, core_ids=list(range(NCORES)),
                               trace=False)
    return _host_finish(res.results, W_out, b_out)


if __name__ == "__main__":
    # quick self-driven run with random inputs
    rng = np.random.default_rng(0)
    ins = {
        "encoder_outputs": rng.standard_normal((B, 64, H)).astype(np.float32),
        "encoder_hidden": rng.standard_normal((B, H)).astype(np.float32),
        "emb": (rng.standard_normal((V, E)) * 0.02).astype(np.float32),
        "W_ih": rng.uniform(-1 / 16, 1 / 16, (3 * H, E)).astype(np.float32),
        "W_hh": rng.uniform(-1 / 22.6, 1 / 22.6, (3 * H, H)).astype(np.float32),
        "b_ih": rng.uniform(-1 / 22.6, 1 / 22.6, (3 * H,)).astype(np.float32),
        "b_hh": rng.uniform(-1 / 22.6, 1 / 22.6, (3 * H,)).astype(np.float32),
        "W_out": rng.uniform(-1 / 22.6, 1 / 22.6, (V, H)).astype(np.float32),
        "b_out": rng.uniform(-1 / 22.6, 1 / 22.6, (V,)).astype(np.float32),
    }
    out = kernel(**ins)
    print([o.shape for o in out])


# revision 23
# speedup vs baseline: 3.6850x; 3.6850x over previous
"""Trainium2 Bass kernel for nn_BeamSearchDecoder (B=32, K=3, H=512, E=256,
V=32000, 32 decode steps), SPMD over 8 NeuronCores.

Key observation (verified against the reference): all K=3 beams start from an
identical state (h0 repeated, same START token, zero scores) and jax.lax.top_k
breaks ties by lower index, so the beam search is exactly greedy decoding with
every beam identical at every step (bitwise).  The kernel therefore runs a
greedy GRU decoder over 32 batch rows and the host replicates beams / builds
the one-hot output.

Distribution: the output projection W_out (32000x512 fp32 = 65.5 MB) is
sharded over the 8 cores by vocab (4000 rows each, SBUF-resident).  Each step
every core computes its logits shard, local top-2 candidates (+ sum-exp for
the log-softmax normalizer), and the 8 per-core candidate rows are combined
with a small AllGather; every core then picks the same next token.

Precision scheme: the big logits matmul runs in float32r (4x faster PE mode,
~1e-4 abs error).  That is accurate enough to FIND the top-2 candidates but
not to ORDER near-ties faithfully vs the fp32 reference, so after the
exchange the two global finalists per row are recomputed exactly (fp32 dot on
the vector engine against W_out rows gathered from DRAM) and the winner is
chosen from the exact values.  The GRU itself stays fp32 so the hidden state
tracks the reference bit-for-bit-ish (~1e-7).

The GRU input projection is folded into a host-precomputed table
  gi_table[v] = relu(emb[v]) @ W_ih.T + b_ih + [b_hh_r, b_hh_z, 0]
gathered per step with an indirect DMA (the n-gate b_hh part must stay inside
the r*(...) term, so it is added separately on device).
"""

import sys

import numpy as np

for _p in ("/opt/trn_rl_repo", "/root/.axon_site/_ro/trn_rl_repo"):
    if _p not in sys.path:
        sys.path.append(_p)

import concourse.bass as bass
import concourse.mybir as mybir
import concourse.tile as tile
from concourse import bacc
from concourse.bass_utils import run_bass_kernel_spmd
from concourse.masks import make_identity

F32 = mybir.dt.float32
F32R = mybir.dt.float32r
F16 = mybir.dt.float16
I32 = mybir.dt.int32
U32 = mybir.dt.uint32
U8 = mybir.dt.uint8
AF = mybir.ActivationFunctionType
OP = mybir.AluOpType
AX = mybir.AxisListType

P = 128
NCORES = 8
B = 32          # batch
K = 3           # beams (degenerate/identical)
H = 512
E = 256
V = 32000
STEPS = 32
START = 1
VSH = V // NCORES          # 4000 vocab per core
QW = VSH // 4              # 1000 per quarter-partition-group
KSUB = H // P              # 4
BIG = 1.0e9
PAY = 5 * B                # payload row: v1 | i1 | v2 | i2 | se


def _build_nc(steps=STEPS, use_cc=True):
    nc = bacc.Bacc("TRN2", target_bir_lowering=False, debug=False,
                   num_devices=NCORES)

    gi_table = nc.dram_tensor("gi_table", [V, 3 * H], F32, kind="ExternalInput")
    w_hhi = nc.dram_tensor("w_hhi", [P, KSUB, 3 * H], F16, kind="ExternalInput")
    w_hhd = nc.dram_tensor("w_hhd", [P, KSUB, 3 * H], F16, kind="ExternalInput")
    w_hlx = nc.dram_tensor("w_hlx", [P, KSUB, 3 * H], F16, kind="ExternalInput")
    b_hhn = nc.dram_tensor("b_hhn", [B, H], F32, kind="ExternalInput")
    wo = nc.dram_tensor("wo", [P, 16, QW], F32, kind="ExternalInput")
    bo = nc.dram_tensor("bo", [B, 4, QW], F32, kind="ExternalInput")
    w_full = nc.dram_tensor("w_full", [V, H], F32, kind="ExternalInput")
    b_outc = nc.dram_tensor("b_outc", [V, 1], F32, kind="ExternalInput")
    h0t = nc.dram_tensor("h0t", [P, KSUB, B], F32, kind="ExternalInput")
    h0row = nc.dram_tensor("h0row", [B, H], F32, kind="ExternalInput")
    vocoff = nc.dram_tensor("vocoff", [P, 1], F32, kind="ExternalInput")
    hbidx = nc.dram_tensor("hbidx", [2 * B, 1], I32, kind="ExternalInput")

    steps_out = nc.dram_tensor("steps_out", [STEPS, B, 8], F32,
                               kind="ExternalOutput")
    toks_out = nc.dram_tensor("toks_out", [STEPS, B, 1], F32,
                              kind="ExternalOutput")
    h_out = nc.dram_tensor("h_out", [B, H], F32, kind="ExternalOutput")

    with tile.TileContext(nc) as tc:
        with tc.tile_pool(name="const", bufs=1) as cp, \
             tc.tile_pool(name="sb", bufs=2) as sb, \
             tc.tile_pool(name="ps", bufs=1, space="PSUM") as ps, \
             tc.tile_pool(name="dr", bufs=2, space="DRAM") as dr:

            # ---- resident constants ----
            w_hhi_sb = cp.tile([P, KSUB, 3 * H], F16)
            nc.sync.dma_start(w_hhi_sb[:], w_hhi[:])
            w_hhd_sb = cp.tile([P, KSUB, 3 * H], F16)
            nc.sync.dma_start(w_hhd_sb[:], w_hhd[:])
            w_hlx_sb = cp.tile([P, KSUB, 3 * H], F16)
            nc.sync.dma_start(w_hlx_sb[:], w_hlx[:])
            # wo loaded via staging chunks and rounded into an F32R tile
            wo_r = cp.tile([P, 16, QW], F32R)
            for i in range(16):
                wstage = sb.tile([P, QW], F32, tag="wstage")
                nc.sync.dma_start(wstage[:], wo[:, i, :])
                nc.vector.tensor_copy(wo_r[:, i, :], wstage[:])
            bo_sb = cp.tile([B, 4, QW], F32)
            nc.sync.dma_start(bo_sb[:], bo[:])
            b_hhn_sb = cp.tile([B, H], F32)
            nc.sync.dma_start(b_hhn_sb[:], b_hhn[:])
            vocoff_sb = cp.tile([P, 1], F32)
            nc.sync.dma_start(vocoff_sb[:], vocoff[:])
            hbidx_sb = cp.tile([2 * B, 1], I32)
            nc.sync.dma_start(hbidx_sb[:], hbidx[:])
            ident = cp.tile([P, P], F32)
            make_identity(nc, ident[:])
            big_sb = cp.tile([P, 1], F32)
            nc.vector.memset(big_sb[:], BIG)
            nbig_sb = cp.tile([P, 1], F32)
            nc.vector.memset(nbig_sb[:], -BIG)

            # ---- state ----
            hT0 = cp.tile([P, KSUB, B], F32, name="hT0")
            nc.sync.dma_start(hT0[:], h0t[:])
            hhi = cp.tile([P, KSUB, B], F16, name="hhi0")
            nc.vector.tensor_copy(hhi[:].rearrange("p k b -> p (k b)"),
                                  hT0[:].rearrange("p k b -> p (k b)"))
            tlo0 = cp.tile([P, KSUB, B], F32, name="tlo0")
            nc.vector.tensor_sub(tlo0[:].rearrange("p k b -> p (k b)"),
                                 hT0[:].rearrange("p k b -> p (k b)"),
                                 hhi[:].rearrange("p k b -> p (k b)"))
            hlox = cp.tile([P, KSUB, B], F16, name="hlox0")
            nc.vector.tensor_scalar(hlox[:].rearrange("p k b -> p (k b)"),
                                    tlo0[:].rearrange("p k b -> p (k b)"),
                                    64.0, None, op0=OP.mult)
            hhid = cp.tile([P, KSUB, B], F16, name="hhid0")
            nc.vector.tensor_scalar(hhid[:].rearrange("p k b -> p (k b)"),
                                    hT0[:].rearrange("p k b -> p (k b)"),
                                    1.0 / 64.0, None, op0=OP.mult)
            hrow = cp.tile([B, H], F32, name="hrow0")
            nc.sync.dma_start(hrow[:], h0row[:])
            tok = cp.tile([B, 1], I32, name="tok0")
            nc.vector.memset(tok[:], START)

            for t in range(steps):
                # 1. gather gi = gi_table[tok]  -> [B, 3H]
                gi = sb.tile([B, 3 * H], F32, tag="gi")
                nc.gpsimd.indirect_dma_start(
                    out=gi[:], out_offset=None,
                    in_=gi_table[:],
                    in_offset=bass.IndirectOffsetOnAxis(ap=tok[:, :1], axis=0),
                )

                # 2. gh matmuls: RZ [B,1024], HN [B,512] (fp32)
                rz_ps = ps.tile([B, 2 * H], F32, tag="rz")
                hn_ps = ps.tile([B, H], F32, tag="hn")
                terms = ((hhi, w_hhi_sb), (hlox, w_hhd_sb), (hhid, w_hlx_sb))
                for ch in range(2):
                    first = True
                    for k in range(KSUB):
                        for hs, ws in terms:
                            nc.tensor.matmul(
                                rz_ps[:, ch * H:(ch + 1) * H],
                                lhsT=hs[:, k, :],
                                rhs=ws[:, k, ch * H:(ch + 1) * H],
                                start=first,
                                stop=(k == KSUB - 1 and hs is hhid))
                            first = False
                first = True
                for k in range(KSUB):
                    for hs, ws in terms:
                        nc.tensor.matmul(
                            hn_ps[:],
                            lhsT=hs[:, k, :],
                            rhs=ws[:, k, 2 * H:3 * H],
                            start=first,
                            stop=(k == KSUB - 1 and hs is hhid))
                        first = False

                # 3. r,z = sigmoid(gh_rz + gi_rz)
                rzs = sb.tile([B, 2 * H], F32, tag="rzs")
                nc.vector.tensor_add(rzs[:], rz_ps[:], gi[:, :2 * H])
                sig = sb.tile([B, 2 * H], F32, tag="sig")
                nc.scalar.activation(sig[:], rzs[:], AF.Sigmoid)

                # 4. n = tanh(gi_n + r * (gh_n + b_hh_n))
                hn2 = sb.tile([B, H], F32, tag="hn2")
                nc.vector.tensor_add(hn2[:], hn_ps[:], b_hhn_sb[:])
                nc.vector.tensor_mul(hn2[:], hn2[:], sig[:, :H])
                nc.vector.tensor_add(hn2[:], hn2[:], gi[:, 2 * H:3 * H])
                n_sb = sb.tile([B, H], F32, tag="n")
                nc.scalar.activation(n_sb[:], hn2[:], AF.Tanh)

                # 5. h_new = n + z * (h - n)
                hrow_new = sb.tile([B, H], F32, tag="hrow")
                nc.vector.tensor_sub(hrow_new[:], hrow[:], n_sb[:])
                nc.vector.tensor_mul(hrow_new[:], hrow_new[:], sig[:, H:2 * H])
                nc.vector.tensor_add(hrow_new[:], hrow_new[:], n_sb[:])

                # stage h to DRAM for the exact-recompute gather
                h_dram = dr.tile([B, H], F32, tag="hdram")
                nc.sync.dma_start(h_dram[:], hrow_new[:])

                # 6. transpose h_new -> hT_new [128, 4, B]
                ht_ps = ps.tile([P, KSUB * B], F32, tag="htp")
                for k in range(KSUB):
                    nc.tensor.transpose(ht_ps[:, k * B:(k + 1) * B],
                                        hrow_new[:, k * P:(k + 1) * P],
                                        ident[:B, :B])
                hT_r = sb.tile([P, KSUB, B], F32R, tag="hTr")
                nc.vector.tensor_copy(hT_r[:].rearrange("p k b -> p (k b)"),
                                      ht_ps[:])
                hhi_n = sb.tile([P, KSUB, B], F16, tag="hhi")
                nc.vector.tensor_copy(hhi_n[:].rearrange("p k b -> p (k b)"),
                                      ht_ps[:])
                tlo = sb.tile([P, KSUB, B], F32, tag="tlo")
                nc.vector.tensor_sub(tlo[:].rearrange("p k b -> p (k b)"),
                                     ht_ps[:],
                                     hhi_n[:].rearrange("p k b -> p (k b)"))
                hlox_n = sb.tile([P, KSUB, B], F16, tag="hlox")
                nc.vector.tensor_scalar(
                    hlox_n[:].rearrange("p k b -> p (k b)"),
                    tlo[:].rearrange("p k b -> p (k b)"),
                    64.0, None, op0=OP.mult)
                hhid_n = sb.tile([P, KSUB, B], F16, tag="hhid")
                nc.vector.tensor_scalar(
                    hhid_n[:].rearrange("p k b -> p (k b)"),
                    ht_ps[:], 1.0 / 64.0, None, op0=OP.mult)

                # 7. logits shard in fp32r, 4 serial chunks of 1000
                #    chunk c covers vocab [c*1000, (c+1)*1000) of this shard
                cand_v = sb.tile([B, 8], F32, tag="candv")
                cand_if = sb.tile([B, 8], F32, tag="candif")
                seall = sb.tile([B, 4], F32, tag="seall")
                for c in range(4):
                    lgc = ps.tile([B, 1024], F32, tag="lgc", bufs=2)
                    for c0, c1 in ((0, H), (H, QW)):
                        for k in range(KSUB):
                            nc.tensor.matmul(
                                lgc[:, c0:c1],
                                lhsT=hT_r[:, k, :],
                                rhs=wo_r[:, c * KSUB + k, c0:c1],
                                start=(k == 0), stop=(k == KSUB - 1))
                    nc.vector.tensor_add(lgc[:, :QW], lgc[:, :QW],
                                         bo_sb[:, c, :])
                    v8c = sb.tile([B, 8], F32, tag="v8c")
                    nc.vector.max(v8c[:], lgc[:, :QW])
                    i8c = sb.tile([B, 8], U32, tag="i8c")
                    nc.vector.max_index(i8c[:], v8c[:], lgc[:, :QW])
                    esc = sb.tile([B, QW], F32, tag="esc")
                    nc.scalar.activation(esc[:], lgc[:, :QW], AF.Exp,
                                         accum_out=seall[:, c:c + 1])
                    nc.vector.tensor_copy(cand_v[:, 2 * c:2 * c + 2],
                                          v8c[:, 0:2])
                    nc.vector.tensor_copy(cand_if[:, 2 * c:2 * c + 2],
                                          i8c[:, 0:2])
                    if c:
                        nc.vector.tensor_scalar(
                            cand_if[:, 2 * c:2 * c + 2],
                            cand_if[:, 2 * c:2 * c + 2],
                            float(c * QW), None, op0=OP.add)
                nc.vector.tensor_scalar(cand_if[:], cand_if[:],
                                        vocoff_sb[:B, 0:1], None, op0=OP.add)

                # 8. payload row per batch: v1 i1 v2 i2 se 0 0 0
                pay5 = sb.tile([B, 8], F32, tag="pay5")
                nc.vector.memset(pay5[:], 0.0)
                nc.vector.tensor_reduce(out=pay5[:, 4:5], in_=seall[:],
                                        op=OP.add, axis=AX.X)

                def top2_cols(valv, idxv, fshape, axis, out, cv1, ci1, cv2,
                              ci2, tg):
                    # top-2 by (value desc, ties lowest idx) over free axes
                    bc = lambda apc: apc[:B, 0:1].to_broadcast(fshape)                         if len(fshape) == 2 else                         apc[:B, 0:1, None].to_broadcast(fshape)
                    bco = lambda apo: apo.to_broadcast(fshape)                         if len(fshape) == 2 else                         apo[:, :, None].to_broadcast(fshape)
                    nc.vector.tensor_reduce(out=out[:, cv1:cv1 + 1],
                                            in_=valv, op=OP.max, axis=axis)
                    eqv = sb.tile(list(fshape), U8, tag=f"eqv{tg}")
                    nc.vector.tensor_tensor(eqv[:], valv,
                                            bco(out[:, cv1:cv1 + 1]),
                                            OP.is_equal)
                    i1s = sb.tile(list(fshape), F32, tag=f"i1s{tg}")
                    nc.vector.select(i1s[:], eqv[:], idxv, bc(big_sb))
                    nc.vector.tensor_reduce(out=out[:, ci1:ci1 + 1],
                                            in_=i1s[:], op=OP.min, axis=axis)
                    eqi = sb.tile(list(fshape), U8, tag=f"eqi{tg}")
                    nc.vector.tensor_tensor(eqi[:], idxv,
                                            bco(out[:, ci1:ci1 + 1]),
                                            OP.is_equal)
                    nc.vector.tensor_tensor(eqi[:], eqi[:], eqv[:],
                                            OP.logical_and)
                    vm = sb.tile(list(fshape), F32, tag=f"vm{tg}")
                    nc.vector.select(vm[:], eqi[:], bc(nbig_sb), valv)
                    nc.vector.tensor_reduce(out=out[:, cv2:cv2 + 1],
                                            in_=vm[:], op=OP.max, axis=axis)
                    eq2v = sb.tile(list(fshape), U8, tag=f"eq2v{tg}")
                    nc.vector.tensor_tensor(eq2v[:], vm[:],
                                            bco(out[:, cv2:cv2 + 1]),
                                            OP.is_equal)
                    i2s = sb.tile(list(fshape), F32, tag=f"i2s{tg}")
                    nc.vector.select(i2s[:], eq2v[:], idxv, bc(big_sb))
                    nc.vector.tensor_reduce(out=out[:, ci2:ci2 + 1],
                                            in_=i2s[:], op=OP.min, axis=axis)

                top2_cols(cand_v[:], cand_if[:], (B, 8), AX.X,
                          pay5, 0, 1, 2, 3, "q")

                # own-core per-step record -> host
                nc.sync.dma_start(steps_out[t], pay5[:])

                if t == steps - 1:
                    break   # host does the final-step selection itself

                # 9. exchange candidate rows
                cc_in = dr.tile([B, 8], F32, tag="ccin")
                cc_out = dr.tile([NCORES, B, 8], F32,
                                 addr_space="Shared" if use_cc else "Local",
                                 tag="ccout")
                nc.sync.dma_start(cc_in[:], pay5[:])
                if use_cc:
                    nc.gpsimd.collective_compute(
                        "AllGather", OP.bypass,
                        replica_groups=[list(range(NCORES))],
                        ins=[cc_in[:].opt()], outs=[cc_out[:].opt()],
                    )
                else:  # timing-only variant: fake the gather locally
                    for cx in range(2):
                        nc.sync.dma_start(
                            cc_out[cx * 4:(cx + 1) * 4],
                            cc_in[None, :, :].to_broadcast([4, B, 8]))
                recv = sb.tile([B, NCORES, 8], F32, tag="recv")
                nc.sync.dma_start(recv[:],
                                  cc_out[:].rearrange("c b f -> b c f"))

                # 10. global top-2 by fp32r value over 16 candidates
                v4 = recv[:, :, 0:4].rearrange("b c (m vi) -> b c m vi", m=2)
                valg = v4[:, :, :, 0]          # [B, 8, 2]
                idxg = v4[:, :, :, 1]
                gpay = sb.tile([B, 4], F32, tag="gpay")   # gv1 gi1 gv2 gi2
                top2_cols(valg, idxg, (B, NCORES, 2), AX.XY,
                          gpay, 0, 1, 2, 3, "g")

                # 11. exact fp32 recompute of the two finalists
                cand_i = sb.tile([2 * B, 1], I32, tag="candi")
                nc.vector.tensor_copy(cand_i[0:B], gpay[:, 1:2])
                nc.vector.tensor_copy(cand_i[B:2 * B], gpay[:, 3:4])
                wcand = sb.tile([2 * B, H], F32, tag="wcand")
                nc.gpsimd.indirect_dma_start(
                    out=wcand[:], out_offset=None, in_=w_full[:],
                    in_offset=bass.IndirectOffsetOnAxis(ap=cand_i[:, :1],
                                                        axis=0))
                bcand = sb.tile([2 * B, 1], F32, tag="bcand")
                nc.gpsimd.indirect_dma_start(
                    out=bcand[:], out_offset=None, in_=b_outc[:],
                    in_offset=bass.IndirectOffsetOnAxis(ap=cand_i[:, :1],
                                                        axis=0))
                hcand = sb.tile([2 * B, H], F32, tag="hcand")
                nc.gpsimd.indirect_dma_start(
                    out=hcand[:], out_offset=None, in_=h_dram[:],
                    in_offset=bass.IndirectOffsetOnAxis(ap=hbidx_sb[:, :1],
                                                        axis=0))
                nc.vector.tensor_mul(wcand[:], wcand[:], hcand[:])
                vex = sb.tile([2 * B, 1], F32, tag="vex")
                nc.vector.tensor_reduce(out=vex[:], in_=wcand[:],
                                        op=OP.add, axis=AX.X)
                nc.vector.tensor_add(vex[:], vex[:], bcand[:])
                # realign (m,b) rows -> per-b columns via a DRAM bounce
                vex_d = dr.tile([2 * B, 1], F32, tag="vexd")
                nc.sync.dma_start(vex_d[:], vex[:])
                vexb = sb.tile([B, 2], F32, tag="vexb")
                nc.sync.dma_start(vexb[:],
                                  vex_d[:].rearrange("(m b) o -> b (m o)",
                                                     m=2))

                # 12. winner: cand2 iff v2 > v1 or (v2 == v1 and i2 < i1)
                gtm = sb.tile([B, 1], U8, tag="gtm")
                nc.vector.tensor_tensor(gtm[:], vexb[:, 1:2], vexb[:, 0:1],
                                        OP.is_gt)
                eqm = sb.tile([B, 1], U8, tag="eqm")
                nc.vector.tensor_tensor(eqm[:], vexb[:, 1:2], vexb[:, 0:1],
                                        OP.is_equal)
                ltm = sb.tile([B, 1], U8, tag="ltm")
                nc.vector.tensor_tensor(ltm[:], gpay[:, 3:4], gpay[:, 1:2],
                                        OP.is_lt)
                nc.vector.tensor_tensor(eqm[:], eqm[:], ltm[:],
                                        OP.logical_and)
                nc.vector.tensor_tensor(gtm[:], gtm[:], eqm[:],
                                        OP.logical_or)
                tokf = sb.tile([B, 1], F32, tag="tokf")
                nc.vector.select(tokf[:], gtm[:], gpay[:, 3:4], gpay[:, 1:2])
                nc.sync.dma_start(toks_out[t], tokf[:])
                tok_new = sb.tile([B, 1], I32, tag="tok")
                nc.vector.tensor_copy(tok_new[:], tokf[:])

                hhi, hlox, hhid = hhi_n, hlox_n, hhid_n
                hrow, tok = hrow_new, tok_new

            nc.sync.dma_start(h_out[:], hrow_new[:])

    nc.compile()
    return nc


_NC_CACHE = None


def _get_nc():
    global _NC_CACHE
    if _NC_CACHE is None:
        _NC_CACHE = _build_nc()
    return _NC_CACHE


def _host_prep(encoder_hidden, emb, W_ih, W_hh, b_ih, b_hh, W_out, b_out):
    emb = np.asarray(emb, np.float32)
    W_ih = np.asarray(W_ih, np.float32)
    W_hh = np.asarray(W_hh, np.float32)
    b_ih = np.asarray(b_ih, np.float32)
    b_hh = np.asarray(b_hh, np.float32)
    W_out = np.ascontiguousarray(np.asarray(W_out, np.float32))
    b_out = np.asarray(b_out, np.float32)
    h0 = np.asarray(encoder_hidden, np.float32)

    bias = b_ih.copy()
    bias[:2 * H] += b_hh[:2 * H]
    gi_table = np.maximum(emb, 0.0).astype(np.float32) @ W_ih.T + bias
    gi_table = np.ascontiguousarray(gi_table, np.float32)

    w_hht = np.ascontiguousarray(
        W_hh.T.reshape(KSUB, P, 3 * H).transpose(1, 0, 2), np.float32)
    w_hhi = w_hht.astype(np.float16)
    w_hhd = (w_hhi.astype(np.float32) / 64.0).astype(np.float16)
    w_hlx = ((w_hht - w_hhi.astype(np.float32)) * 64.0).astype(np.float16)
    b_hhn = np.ascontiguousarray(
        np.broadcast_to(b_hh[2 * H:], (B, H)), np.float32)
    h0t = np.ascontiguousarray(
        h0.T.reshape(KSUB, P, B).transpose(1, 0, 2), np.float32)

    common = {
        "gi_table": gi_table,
        "w_hhi": np.ascontiguousarray(w_hhi),
        "w_hhd": np.ascontiguousarray(w_hhd),
        "w_hlx": np.ascontiguousarray(w_hlx),
        "b_hhn": b_hhn,
        "w_full": W_out,
        "b_outc": np.ascontiguousarray(b_out[:, None]),
        "h0t": h0t,
        "h0row": np.ascontiguousarray(h0, np.float32),
        "hbidx": np.ascontiguousarray(
            (np.arange(2 * B) % B).astype(np.int32)[:, None]),
    }
    in_maps = []
    for c in range(NCORES):
        wc = W_out[c * VSH:(c + 1) * VSH]                      # [4000, 512]
        wo = np.ascontiguousarray(
            wc.T.reshape(KSUB, P, 4, QW).transpose(1, 2, 0, 3)
            .reshape(P, 16, QW), np.float32)
        bc = b_out[c * VSH:(c + 1) * VSH].reshape(4, QW)
        bo = np.ascontiguousarray(
            np.broadcast_to(bc[None], (B, 4, QW)), np.float32)
        vocoff = np.full((P, 1), float(c * VSH), np.float32)
        m = dict(common)
        m.update({"wo": wo, "bo": bo,
                  "vocoff": np.ascontiguousarray(vocoff)})
        in_maps.append(m)
    return in_maps


def _host_finish(results, W_out, b_out):
    """Rebuild (decoded, h, scores) from per-core per-step candidates."""
    W_out = np.asarray(W_out, np.float64)
    b_out = np.asarray(b_out, np.float64)
    h_final = results[0]["h_out"]                          # [B, H] f32
    pays = np.stack([r["steps_out"] for r in results])     # [C, T, B, 8]
    v1 = pays[:, :, :, 0].astype(np.float64)               # [C, T, B]
    i1 = pays[:, :, :, 1].astype(np.int64)
    v2 = pays[:, :, :, 2].astype(np.float64)
    i2 = pays[:, :, :, 3].astype(np.int64)
    sume = pays[:, :, :, 4].astype(np.float64)
    toks_dev = results[0]["toks_out"][:, :, 0].astype(np.int64)  # [T, B]

    cvals = np.concatenate([v1, v2], axis=0)               # [2C, T, B]
    cidx = np.concatenate([i1, i2], axis=0)

    tok = np.zeros((STEPS, B), np.int64)
    tok[:STEPS - 1] = toks_dev[:STEPS - 1]
    # final step: select among the exchanged candidates with exact math
    t = STEPS - 1
    for b in range(B):
        ci = cidx[:, t, b]
        ex = W_out[ci] @ h_final[b].astype(np.float64) + b_out[ci]
        order = np.lexsort((ci, -ex))
        tok[t, b] = ci[order[0]]

    # scores: fp32r candidate value of the chosen token each step
    lse = np.log(sume.sum(axis=0))                         # [T, B]
    hit = (cidx == tok[None]).astype(np.float64)           # [2C, T, B]
    vchosen = (cvals * hit).sum(axis=0)
    cum = (vchosen - lse).sum(axis=0)                      # [B]

    seqs = np.full((B, STEPS + 1), START, np.int64)
    seqs[:, 1:] = tok.T                                    # greedy: old = id
    decoded = np.zeros((B, STEPS + 1, V), np.float32)
    bi = np.repeat(np.arange(B), STEPS + 1)
    ti = np.tile(np.arange(STEPS + 1), B)
    decoded[bi, ti, seqs.reshape(-1)] = 1.0

    h = np.repeat(h_final[:, None, :], K, axis=1).astype(np.float32)
    scores = np.repeat(cum.astype(np.float32)[:, None], K, axis=1)
    return decoded, h, scores


def kernel(encoder_outputs, encoder_hidden, emb, W_ih, W_hh, b_ih, b_hh,
           W_out, b_out):
    nc = _get_nc()
    in_maps = _host_prep(encoder_hidden, emb, W_ih, W_hh, b_ih, b_hh,
                         W_out, b_out)
    res = run_bass_kernel_spmd(nc, in_maps, core_ids=list(range(NCORES)),
                               trace=False)
    return _host_finish(res.results, W_out, b_out)


if __name__ == "__main__":
    # quick self-driven run with random inputs
    rng = np.random.default_rng(0)
    ins = {
        "encoder_outputs": rng.standard_normal((B, 64, H)).astype(np.float32),
        "encoder_hidden": rng.standard_normal((B, H)).astype(np.float32),
        "emb": (rng.standard_normal((V, E)) * 0.02).astype(np.float32),
        "W_ih": rng.uniform(-1 / 16, 1 / 16, (3 * H, E)).astype(np.float32),
        "W_hh": rng.uniform(-1 / 22.6, 1 / 22.6, (3 * H, H)).astype(np.float32),
        "b_ih": rng.uniform(-1 / 22.6, 1 / 22.6, (3 * H,)).astype(np.float32),
        "b_hh": rng.uniform(-1 / 22.6, 1 / 22.6, (3 * H,)).astype(np.float32),
        "W_out": rng.uniform(-1 / 22.6, 1 / 22.6, (V, H)).astype(np.float32),
        "b_out": rng.uniform(-1 / 22.6, 1 / 22.6, (V,)).astype(np.float32),
    }
    out = kernel(**ins)
    print([o.shape for o in out])


# revision 25
# speedup vs baseline: 19.2650x; 5.2280x over previous
"""Trainium2 Bass kernel for nn_BeamSearchDecoder (B=32, K=3, H=512, E=256,
V=32000, 32 decode steps), SPMD over 8 NeuronCores.

Key observation (verified against the reference): all K=3 beams start from an
identical state (h0 repeated, same START token, zero scores) and jax.lax.top_k
breaks ties by lower index, so the beam search is exactly greedy decoding with
every beam identical at every step (bitwise).  The kernel therefore runs a
greedy GRU decoder over 32 batch rows and the host replicates beams / builds
the one-hot output.

Distribution: the output projection W_out (32000x512 fp32 = 65.5 MB) is
sharded over the 8 cores by vocab (4000 rows each, SBUF-resident).  Each step
every core computes its logits shard, local top-2 candidates (+ sum-exp for
the log-softmax normalizer), and the 8 per-core candidate rows are combined
with a small AllGather; every core then picks the same next token.

Precision scheme: the big logits matmul runs in float32r (4x faster PE mode,
~1e-4 abs error).  That is accurate enough to FIND the top-2 candidates but
not to ORDER near-ties faithfully vs the fp32 reference, so after the
exchange the two global finalists per row are recomputed exactly (fp32 dot on
the vector engine against W_out rows gathered from DRAM) and the winner is
chosen from the exact values.  The GRU itself stays fp32 so the hidden state
tracks the reference bit-for-bit-ish (~1e-7).

The GRU input projection is folded into a host-precomputed table
  gi_table[v] = relu(emb[v]) @ W_ih.T + b_ih + [b_hh_r, b_hh_z, 0]
gathered per step with an indirect DMA (the n-gate b_hh part must stay inside
the r*(...) term, so it is added separately on device).
"""

import sys

import numpy as np

for _p in ("/opt/trn_rl_repo", "/root/.axon_site/_ro/trn_rl_repo"):
    if _p not in sys.path:
        sys.path.append(_p)

import concourse.bass as bass
import concourse.mybir as mybir
import concourse.tile as tile
from concourse import bacc
from concourse.bass_utils import run_bass_kernel_spmd
from concourse.masks import make_identity

F32 = mybir.dt.float32
F32R = mybir.dt.float32r
F16 = mybir.dt.float16
I32 = mybir.dt.int32
U32 = mybir.dt.uint32
U8 = mybir.dt.uint8
AF = mybir.ActivationFunctionType
OP = mybir.AluOpType
AX = mybir.AxisListType

P = 128
NCORES = 8
B = 32          # batch
K = 3           # beams (degenerate/identical)
H = 512
E = 256
V = 32000
STEPS = 32
START = 1
VSH = V // NCORES          # 4000 vocab per core
QW = VSH // 4              # 1000 per quarter-partition-group
KSUB = H // P              # 4
BIG = 1.0e9
PAY = 5 * B                # payload row: v1 | i1 | v2 | i2 | se


def _build_nc(steps=STEPS, use_cc=True):
    nc = bacc.Bacc("TRN2", target_bir_lowering=False, debug=False,
                   num_devices=NCORES)

    gi_table = nc.dram_tensor("gi_table", [V, 3 * H], F32, kind="ExternalInput")
    w_hhi = nc.dram_tensor("w_hhi", [P, KSUB, 3 * H], F16, kind="ExternalInput")
    w_hhd = nc.dram_tensor("w_hhd", [P, KSUB, 3 * H], F16, kind="ExternalInput")
    w_hlx = nc.dram_tensor("w_hlx", [P, KSUB, 3 * H], F16, kind="ExternalInput")
    b_hhn = nc.dram_tensor("b_hhn", [B, H], F32, kind="ExternalInput")
    wo = nc.dram_tensor("wo", [P, 16, QW], F32, kind="ExternalInput")
    bo = nc.dram_tensor("bo", [B, 4, QW], F32, kind="ExternalInput")
    w_full = nc.dram_tensor("w_full", [V, H], F32, kind="ExternalInput")
    b_outc = nc.dram_tensor("b_outc", [V, 1], F32, kind="ExternalInput")
    h0t = nc.dram_tensor("h0t", [P, KSUB, B], F32, kind="ExternalInput")
    h0row = nc.dram_tensor("h0row", [B, H], F32, kind="ExternalInput")
    vocoff = nc.dram_tensor("vocoff", [P, 1], F32, kind="ExternalInput")
    hbidx = nc.dram_tensor("hbidx", [2 * B, 1], I32, kind="ExternalInput")

    steps_out = nc.dram_tensor("steps_out", [STEPS, B, 8], F32,
                               kind="ExternalOutput")
    toks_out = nc.dram_tensor("toks_out", [STEPS, B, 1], F32,
                              kind="ExternalOutput")
    h_out = nc.dram_tensor("h_out", [B, H], F32, kind="ExternalOutput")

    with tile.TileContext(nc) as tc:
        with tc.tile_pool(name="const", bufs=1) as cp, \
             tc.tile_pool(name="sb", bufs=2) as sb, \
             tc.tile_pool(name="ps", bufs=1, space="PSUM") as ps, \
             tc.tile_pool(name="dr", bufs=2, space="DRAM") as dr:

            # ---- resident constants ----
            w_hhi_sb = cp.tile([P, KSUB, 3 * H], F16)
            nc.sync.dma_start(w_hhi_sb[:], w_hhi[:])
            w_hhd_sb = cp.tile([P, KSUB, 3 * H], F16)
            nc.sync.dma_start(w_hhd_sb[:], w_hhd[:])
            w_hlx_sb = cp.tile([P, KSUB, 3 * H], F16)
            nc.sync.dma_start(w_hlx_sb[:], w_hlx[:])
            # wo loaded via staging chunks and rounded into an F32R tile
            wo_r = cp.tile([P, 16, QW], F32R)
            for i in range(16):
                wstage = sb.tile([P, QW], F32, tag="wstage")
                nc.sync.dma_start(wstage[:], wo[:, i, :])
                nc.vector.tensor_copy(wo_r[:, i, :], wstage[:])
            bo_sb = cp.tile([B, 4, QW], F32)
            nc.sync.dma_start(bo_sb[:], bo[:])
            b_hhn_sb = cp.tile([B, H], F32)
            nc.sync.dma_start(b_hhn_sb[:], b_hhn[:])
            vocoff_sb = cp.tile([P, 1], F32)
            nc.sync.dma_start(vocoff_sb[:], vocoff[:])
            hbidx_sb = cp.tile([2 * B, 1], I32)
            nc.sync.dma_start(hbidx_sb[:], hbidx[:])
            ident = cp.tile([P, P], F32)
            make_identity(nc, ident[:])
            big_sb = cp.tile([P, 1], F32)
            nc.vector.memset(big_sb[:], BIG)
            nbig_sb = cp.tile([P, 1], F32)
            nc.vector.memset(nbig_sb[:], -BIG)

            # ---- state ----
            hT0 = cp.tile([P, KSUB, B], F32, name="hT0")
            nc.sync.dma_start(hT0[:], h0t[:])
            hhi = cp.tile([P, KSUB, B], F16, name="hhi0")
            nc.vector.tensor_copy(hhi[:].rearrange("p k b -> p (k b)"),
                                  hT0[:].rearrange("p k b -> p (k b)"))
            tlo0 = cp.tile([P, KSUB, B], F32, name="tlo0")
            nc.vector.tensor_sub(tlo0[:].rearrange("p k b -> p (k b)"),
                                 hT0[:].rearrange("p k b -> p (k b)"),
                                 hhi[:].rearrange("p k b -> p (k b)"))
            hlox = cp.tile([P, KSUB, B], F16, name="hlox0")
            nc.vector.tensor_scalar(hlox[:].rearrange("p k b -> p (k b)"),
                                    tlo0[:].rearrange("p k b -> p (k b)"),
                                    64.0, None, op0=OP.mult)
            hhid = cp.tile([P, KSUB, B], F16, name="hhid0")
            nc.vector.tensor_scalar(hhid[:].rearrange("p k b -> p (k b)"),
                                    hT0[:].rearrange("p k b -> p (k b)"),
                                    1.0 / 64.0, None, op0=OP.mult)
            hrow = cp.tile([B, H], F32, name="hrow0")
            nc.sync.dma_start(hrow[:], h0row[:])
            tok = cp.tile([B, 1], I32, name="tok0")
            nc.vector.memset(tok[:], START)

            for t in range(steps):
                # 1. gather gi = gi_table[tok]  -> [B, 3H]
                gi = sb.tile([B, 3 * H], F32, tag="gi")
                nc.gpsimd.indirect_dma_start(
                    out=gi[:], out_offset=None,
                    in_=gi_table[:],
                    in_offset=bass.IndirectOffsetOnAxis(ap=tok[:, :1], axis=0),
                )

                # 2. gh matmuls: RZ [B,1024], HN [B,512] (fp32)
                rz_ps = ps.tile([B, 2 * H], F32, tag="rz")
                hn_ps = ps.tile([B, H], F32, tag="hn")
                terms = ((hhi, w_hhi_sb), (hlox, w_hhd_sb), (hhid, w_hlx_sb))
                for ch in range(2):
                    first = True
                    for k in range(KSUB):
                        for hs, ws in terms:
                            nc.tensor.matmul(
                                rz_ps[:, ch * H:(ch + 1) * H],
                                lhsT=hs[:, k, :],
                                rhs=ws[:, k, ch * H:(ch + 1) * H],
                                start=first,
                                stop=(k == KSUB - 1 and hs is hhid))
                            first = False
                first = True
                for k in range(KSUB):
                    for hs, ws in terms:
                        nc.tensor.matmul(
                            hn_ps[:],
                            lhsT=hs[:, k, :],
                            rhs=ws[:, k, 2 * H:3 * H],
                            start=first,
                            stop=(k == KSUB - 1 and hs is hhid))
                        first = False

                # 3. r,z = sigmoid(gh_rz + gi_rz)
                rzs = sb.tile([B, 2 * H], F32, tag="rzs")
                nc.vector.tensor_add(rzs[:], rz_ps[:], gi[:, :2 * H])
                sig = sb.tile([B, 2 * H], F32, tag="sig")
                nc.scalar.activation(sig[:], rzs[:], AF.Sigmoid)

                # 4. n = tanh(gi_n + r * (gh_n + b_hh_n))
                hn2 = sb.tile([B, H], F32, tag="hn2")
                nc.vector.tensor_add(hn2[:], hn_ps[:], b_hhn_sb[:])
                nc.vector.tensor_mul(hn2[:], hn2[:], sig[:, :H])
                nc.vector.tensor_add(hn2[:], hn2[:], gi[:, 2 * H:3 * H])
                n_sb = sb.tile([B, H], F32, tag="n")
                nc.scalar.activation(n_sb[:], hn2[:], AF.Tanh)

                # 5. h_new = n + z * (h - n)
                hrow_new = sb.tile([B, H], F32, tag="hrow")
                nc.vector.tensor_sub(hrow_new[:], hrow[:], n_sb[:])
                nc.vector.tensor_mul(hrow_new[:], hrow_new[:], sig[:, H:2 * H])
                nc.vector.tensor_add(hrow_new[:], hrow_new[:], n_sb[:])

                # stage h to DRAM for the exact-recompute gather
                h_dram = dr.tile([B, H], F32, tag="hdram")
                nc.sync.dma_start(h_dram[:], hrow_new[:])

                # 6. transpose h_new -> hT_new [128, 4, B]
                ht_ps = ps.tile([P, KSUB * B], F32, tag="htp")
                for k in range(KSUB):
                    nc.tensor.transpose(ht_ps[:, k * B:(k + 1) * B],
                                        hrow_new[:, k * P:(k + 1) * P],
                                        ident[:B, :B])
                hT_r = sb.tile([P, KSUB, B], F32R, tag="hTr")
                nc.vector.tensor_copy(hT_r[:].rearrange("p k b -> p (k b)"),
                                      ht_ps[:])
                hhi_n = sb.tile([P, KSUB, B], F16, tag="hhi")
                nc.vector.tensor_copy(hhi_n[:].rearrange("p k b -> p (k b)"),
                                      ht_ps[:])
                tlo = sb.tile([P, KSUB, B], F32, tag="tlo")
                nc.vector.tensor_sub(tlo[:].rearrange("p k b -> p (k b)"),
                                     ht_ps[:],
                                     hhi_n[:].rearrange("p k b -> p (k b)"))
                hlox_n = sb.tile([P, KSUB, B], F16, tag="hlox")
                nc.vector.tensor_scalar(
                    hlox_n[:].rearrange("p k b -> p (k b)"),
                    tlo[:].rearrange("p k b -> p (k b)"),
                    64.0, None, op0=OP.mult)
                hhid_n = sb.tile([P, KSUB, B], F16, tag="hhid")
                nc.vector.tensor_scalar(
                    hhid_n[:].rearrange("p k b -> p (k b)"),
                    ht_ps[:], 1.0 / 64.0, None, op0=OP.mult)

                # 7. logits shard in fp32r, 4 serial chunks of 1000
                #    chunk c covers vocab [c*1000, (c+1)*1000) of this shard
                cand_v = sb.tile([B, 8], F32, tag="candv")
                cand_if = sb.tile([B, 8], F32, tag="candif")
                seall = sb.tile([B, 4], F32, tag="seall")
                for c in range(4):
                    lgc = ps.tile([B, 1024], F32, tag="lgc", bufs=2)
                    for c0, c1 in ((0, H), (H, QW)):
                        for k in range(KSUB):
                            nc.tensor.matmul(
                                lgc[:, c0:c1],
                                lhsT=hT_r[:, k, :],
                                rhs=wo_r[:, c * KSUB + k, c0:c1],
                                start=(k == 0), stop=(k == KSUB - 1))
                    nc.vector.tensor_add(lgc[:, :QW], lgc[:, :QW],
                                         bo_sb[:, c, :])
                    v8c = sb.tile([B, 8], F32, tag="v8c")
                    nc.vector.max(v8c[:], lgc[:, :QW])
                    i8c = sb.tile([B, 8], U32, tag="i8c")
                    nc.vector.max_index(i8c[:], v8c[:], lgc[:, :QW])
                    esc = sb.tile([B, QW], F32, tag="esc")
                    nc.scalar.activation(esc[:], lgc[:, :QW], AF.Exp,
                                         accum_out=seall[:, c:c + 1])
                    nc.vector.tensor_copy(cand_v[:, 2 * c:2 * c + 2],
                                          v8c[:, 0:2])
                    nc.vector.tensor_copy(cand_if[:, 2 * c:2 * c + 2],
                                          i8c[:, 0:2])
                    if c:
                        nc.vector.tensor_scalar(
                            cand_if[:, 2 * c:2 * c + 2],
                            cand_if[:, 2 * c:2 * c + 2],
                            float(c * QW), None, op0=OP.add)
                nc.vector.tensor_scalar(cand_if[:], cand_if[:],
                                        vocoff_sb[:B, 0:1], None, op0=OP.add)

                # 8. payload row per batch: v1 i1 v2 i2 se 0 0 0
                pay5 = sb.tile([B, 8], F32, tag="pay5")
                nc.vector.memset(pay5[:], 0.0)
                nc.vector.tensor_reduce(out=pay5[:, 4:5], in_=seall[:],
                                        op=OP.add, axis=AX.X)

                def top2_cols(valv, idxv, fshape, axis, out, cv1, ci1, cv2,
                              ci2, tg):
                    # top-2 by (value desc, ties lowest idx) over free axes
                    bc = lambda apc: apc[:B, 0:1].to_broadcast(fshape)                         if len(fshape) == 2 else                         apc[:B, 0:1, None].to_broadcast(fshape)
                    bco = lambda apo: apo.to_broadcast(fshape)                         if len(fshape) == 2 else                         apo[:, :, None].to_broadcast(fshape)
                    nc.vector.tensor_reduce(out=out[:, cv1:cv1 + 1],
                                            in_=valv, op=OP.max, axis=axis)
                    eqv = sb.tile(list(fshape), U8, tag=f"eqv{tg}")
                    nc.vector.tensor_tensor(eqv[:], valv,
                                            bco(out[:, cv1:cv1 + 1]),
                                            OP.is_equal)
                    i1s = sb.tile(list(fshape), F32, tag=f"i1s{tg}")
                    nc.vector.select(i1s[:], eqv[:], idxv, bc(big_sb))
                    nc.vector.tensor_reduce(out=out[:, ci1:ci1 + 1],
                                            in_=i1s[:], op=OP.min, axis=axis)
                    eqi = sb.tile(list(fshape), U8, tag=f"eqi{tg}")
                    nc.vector.tensor_tensor(eqi[:], idxv,
                                            bco(out[:, ci1:ci1 + 1]),
                                            OP.is_equal)
                    nc.vector.tensor_tensor(eqi[:], eqi[:], eqv[:],
                                            OP.logical_and)
                    vm = sb.tile(list(fshape), F32, tag=f"vm{tg}")
                    nc.vector.select(vm[:], eqi[:], bc(nbig_sb), valv)
                    nc.vector.tensor_reduce(out=out[:, cv2:cv2 + 1],
                                            in_=vm[:], op=OP.max, axis=axis)
                    eq2v = sb.tile(list(fshape), U8, tag=f"eq2v{tg}")
                    nc.vector.tensor_tensor(eq2v[:], vm[:],
                                            bco(out[:, cv2:cv2 + 1]),
                                            OP.is_equal)
                    i2s = sb.tile(list(fshape), F32, tag=f"i2s{tg}")
                    nc.vector.select(i2s[:], eq2v[:], idxv, bc(big_sb))
                    nc.vector.tensor_reduce(out=out[:, ci2:ci2 + 1],
                                            in_=i2s[:], op=OP.min, axis=axis)

                top2_cols(cand_v[:], cand_if[:], (B, 8), AX.X,
                          pay5, 0, 1, 2, 3, "q")

                # own-core per-step record -> host
                nc.sync.dma_start(steps_out[t], pay5[:])

                if t == steps - 1:
                    break   # host does the final-step selection itself

                # 9. exchange candidate rows
                cc_in = dr.tile([B, 8], F32, tag="ccin")
                cc_out = dr.tile([NCORES, B, 8], F32,
                                 addr_space="Shared" if use_cc else "Local",
                                 tag="ccout")
                nc.sync.dma_start(cc_in[:], pay5[:])
                if use_cc:
                    nc.gpsimd.collective_compute(
                        "AllGather", OP.bypass,
                        replica_groups=[list(range(NCORES))],
                        ins=[cc_in[:].opt()], outs=[cc_out[:].opt()],
                    )
                else:  # timing-only variant: fake the gather locally
                    for cx in range(2):
                        nc.sync.dma_start(
                            cc_out[cx * 4:(cx + 1) * 4],
                            cc_in[None, :, :].to_broadcast([4, B, 8]))
                recv = sb.tile([B, NCORES, 8], F32, tag="recv")
                nc.sync.dma_start(recv[:],
                                  cc_out[:].rearrange("c b f -> b c f"))

                # 10. global top-2 by fp32r value over 16 candidates
                v4 = recv[:, :, 0:4].rearrange("b c (m vi) -> b c m vi", m=2)
                valg = v4[:, :, :, 0]          # [B, 8, 2]
                idxg = v4[:, :, :, 1]
                gpay = sb.tile([B, 4], F32, tag="gpay")   # gv1 gi1 gv2 gi2
                top2_cols(valg, idxg, (B, NCORES, 2), AX.XY,
                          gpay, 0, 1, 2, 3, "g")

                # 11. exact fp32 recompute of the two finalists
                cand_i = sb.tile([2 * B, 1], I32, tag="candi")
                nc.vector.tensor_copy(cand_i[0:B], gpay[:, 1:2])
                nc.vector.tensor_copy(cand_i[B:2 * B], gpay[:, 3:4])
                wcand = sb.tile([2 * B, H], F32, tag="wcand")
                nc.gpsimd.indirect_dma_start(
                    out=wcand[:], out_offset=None, in_=w_full[:],
                    in_offset=bass.IndirectOffsetOnAxis(ap=cand_i[:, :1],
                                                        axis=0))
                bcand = sb.tile([2 * B, 1], F32, tag="bcand")
                nc.gpsimd.indirect_dma_start(
                    out=bcand[:], out_offset=None, in_=b_outc[:],
                    in_offset=bass.IndirectOffsetOnAxis(ap=cand_i[:, :1],
                                                        axis=0))
                hcand = sb.tile([2 * B, H], F32, tag="hcand")
                nc.gpsimd.indirect_dma_start(
                    out=hcand[:], out_offset=None, in_=h_dram[:],
                    in_offset=bass.IndirectOffsetOnAxis(ap=hbidx_sb[:, :1],
                                                        axis=0))
                nc.vector.tensor_mul(wcand[:], wcand[:], hcand[:])
                vex = sb.tile([2 * B, 1], F32, tag="vex")
                nc.vector.tensor_reduce(out=vex[:], in_=wcand[:],
                                        op=OP.add, axis=AX.X)
                nc.vector.tensor_add(vex[:], vex[:], bcand[:])
                # realign (m,b) rows -> per-b columns via a DRAM bounce
                vex_d = dr.tile([2 * B, 1], F32, tag="vexd")
                nc.sync.dma_start(vex_d[:], vex[:])
                vexb = sb.tile([B, 2], F32, tag="vexb")
                nc.sync.dma_start(vexb[:],
                                  vex_d[:].rearrange("(m b) o -> b (m o)",
                                                     m=2))

                # 12. winner: cand2 iff v2 > v1 or (v2 == v1 and i2 < i1)
                gtm = sb.tile([B, 1], U8, tag="gtm")
                nc.vector.tensor_tensor(gtm[:], vexb[:, 1:2], vexb[:, 0:1],
                                        OP.is_gt)
                eqm = sb.tile([B, 1], U8, tag="eqm")
                nc.vector.tensor_tensor(eqm[:], vexb[:, 1:2], vexb[:, 0:1],
                                        OP.is_equal)
                ltm = sb.tile([B, 1], U8, tag="ltm")
                nc.vector.tensor_tensor(ltm[:], gpay[:, 3:4], gpay[:, 1:2],
                                        OP.is_lt)
                nc.vector.tensor_tensor(eqm[:], eqm[:], ltm[:],
                                        OP.logical_and)
                nc.vector.tensor_tensor(gtm[:], gtm[:], eqm[:],
                                        OP.logical_or)
                tokf = sb.tile([B, 1], F32, tag="tokf")
                nc.vector.select(tokf[:], gtm[:], gpay[:, 3:4], gpay[:, 1:2])
                nc.sync.dma_start(toks_out[t], tokf[:])
                tok_new = sb.tile([B, 1], I32, tag="tok")
                nc.vector.tensor_copy(tok_new[:], tokf[:])

                hhi, hlox, hhid = hhi_n, hlox_n, hhid_n
                hrow, tok = hrow_new, tok_new

            nc.sync.dma_start(h_out[:], hrow_new[:])

    nc.compile()
    return nc


_NC_CACHE = None


def _get_nc():
    global _NC_CACHE
    if _NC_CACHE is None:
        _NC_CACHE = _build_nc()
    return _NC_CACHE


def _host_prep(encoder_hidden, emb, W_ih, W_hh, b_ih, b_hh, W_out, b_out):
    emb = np.asarray(emb, np.float32)
    W_ih = np.asarray(W_ih, np.float32)
    W_hh = np.asarray(W_hh, np.float32)
    b_ih = np.asarray(b_ih, np.float32)
    b_hh = np.asarray(b_hh, np.float32)
    W_out = np.ascontiguousarray(np.asarray(W_out, np.float32))
    b_out = np.asarray(b_out, np.float32)
    h0 = np.asarray(encoder_hidden, np.float32)

    bias = b_ih.copy()
    bias[:2 * H] += b_hh[:2 * H]
    gi_table = np.maximum(emb, 0.0).astype(np.float32) @ W_ih.T + bias
    gi_table = np.ascontiguousarray(gi_table, np.float32)

    w_hht = np.ascontiguousarray(
        W_hh.T.reshape(KSUB, P, 3 * H).transpose(1, 0, 2), np.float32)
    w_hhi = w_hht.astype(np.float16)
    w_hhd = (w_hhi.astype(np.float32) / 64.0).astype(np.float16)
    w_hlx = ((w_hht - w_hhi.astype(np.float32)) * 64.0).astype(np.float16)
    b_hhn = np.ascontiguousarray(
        np.broadcast_to(b_hh[2 * H:], (B, H)), np.float32)
    h0t = np.ascontiguousarray(
        h0.T.reshape(KSUB, P, B).transpose(1, 0, 2), np.float32)

    common = {
        "gi_table": gi_table,
        "w_hhi": np.ascontiguousarray(w_hhi),
        "w_hhd": np.ascontiguousarray(w_hhd),
        "w_hlx": np.ascontiguousarray(w_hlx),
        "b_hhn": b_hhn,
        "w_full": W_out,
        "b_outc": np.ascontiguousarray(b_out[:, None]),
        "h0t": h0t,
        "h0row": np.ascontiguousarray(h0, np.float32),
        "hbidx": np.ascontiguousarray(
            (np.arange(2 * B) % B).astype(np.int32)[:, None]),
    }
    in_maps = []
    for c in range(NCORES):
        wc = W_out[c * VSH:(c + 1) * VSH]                      # [4000, 512]
        wo = np.ascontiguousarray(
            wc.T.reshape(KSUB, P, 4, QW).transpose(1, 2, 0, 3)
            .reshape(P, 16, QW), np.float32)
        bc = b_out[c * VSH:(c + 1) * VSH].reshape(4, QW)
        bo = np.ascontiguousarray(
            np.broadcast_to(bc[None], (B, 4, QW)), np.float32)
        vocoff = np.full((P, 1), float(c * VSH), np.float32)
        m = dict(common)
        m.update({"wo": wo, "bo": bo,
                  "vocoff": np.ascontiguousarray(vocoff)})
        in_maps.append(m)
    return in_maps


def _host_finish(results, W_out, b_out):
    """Rebuild (decoded, h, scores) from per-core per-step candidates."""
    W_out = np.asarray(W_out, np.float64)
    b_out = np.asarray(b_out, np.float64)
    h_final = results[0]["h_out"]                          # [B, H] f32
    pays = np.stack([r["steps_out"] for r in results])     # [C, T, B, 8]
    v1 = pays[:, :, :, 0].astype(np.float64)               # [C, T, B]
    i1 = pays[:, :, :, 1].astype(np.int64)
    v2 = pays[:, :, :, 2].astype(np.float64)
    i2 = pays[:, :, :, 3].astype(np.int64)
    sume = pays[:, :, :, 4].astype(np.float64)
    toks_dev = results[0]["toks_out"][:, :, 0].astype(np.int64)  # [T, B]

    cvals = np.concatenate([v1, v2], axis=0)               # [2C, T, B]
    cidx = np.concatenate([i1, i2], axis=0)

    tok = np.zeros((STEPS, B), np.int64)
    tok[:STEPS - 1] = toks_dev[:STEPS - 1]
    # final step: select among the exchanged candidates with exact math
    t = STEPS - 1
    for b in range(B):
        ci = cidx[:, t, b]
        ex = W_out[ci] @ h_final[b].astype(np.float64) + b_out[ci]
        order = np.lexsort((ci, -ex))
        tok[t, b] = ci[order[0]]

    # scores: fp32r candidate value of the chosen token each step
    lse = np.log(sume.sum(axis=0))                         # [T, B]
    hit = (cidx == tok[None]).astype(np.float64)           # [2C, T, B]
    vchosen = (cvals * hit).sum(axis=0)
    cum = (vchosen - lse).sum(axis=0)                      # [B]

    seqs = np.full((B, STEPS + 1), START, np.int64)
    seqs[:, 1:] = tok.T                                    # greedy: old = id
    decoded = np.zeros((B, STEPS + 1, V), np.float32)
    bi = np.repeat(np.arange(B), STEPS + 1)
    ti = np.tile(np.arange(STEPS + 1), B)
    decoded[bi, ti, seqs.reshape(-1)] = 1.0

    h = np.repeat(h_final[:, None, :], K, axis=1).astype(np.float32)
    scores = np.repeat(cum.astype(np.float32)[:, None], K, axis=1)
    return decoded, h, scores


def kernel(encoder_outputs, encoder_hidden, emb, W_ih, W_hh, b_ih, b_hh,
           W_out, b_out):
    nc = _get_nc()
    in_maps = _host_prep(encoder_hidden, emb, W_ih, W_hh, b_ih, b_hh,
                         W_out, b_out)
    res = run_bass_kernel_spmd(nc, in_maps, core_ids=list(range(NCORES)),
                               trace=False)
    return _host_finish(res.results, W_out, b_out)


if __name__ == "__main__":
    # quick self-driven run with random inputs
    rng = np.random.default_rng(0)
    ins = {
        "encoder_outputs": rng.standard_normal((B, 64, H)).astype(np.float32),
        "encoder_hidden": rng.standard_normal((B, H)).astype(np.float32),
        "emb": (rng.standard_normal((V, E)) * 0.02).astype(np.float32),
        "W_ih": rng.uniform(-1 / 16, 1 / 16, (3 * H, E)).astype(np.float32),
        "W_hh": rng.uniform(-1 / 22.6, 1 / 22.6, (3 * H, H)).astype(np.float32),
        "b_ih": rng.uniform(-1 / 22.6, 1 / 22.6, (3 * H,)).astype(np.float32),
        "b_hh": rng.uniform(-1 / 22.6, 1 / 22.6, (3 * H,)).astype(np.float32),
        "W_out": rng.uniform(-1 / 22.6, 1 / 22.6, (V, H)).astype(np.float32),
        "b_out": rng.uniform(-1 / 22.6, 1 / 22.6, (V,)).astype(np.float32),
    }
    out = kernel(**ins)
    print([o.shape for o in out])
